# revision 25
# baseline (speedup 1.0000x reference)
"""Trainium2 Bass kernel for nn_CNN_25744033972549.

The reference network is three *linear* stages (conv k=10 pad=9, conv k=20
pad=19, sliding-window FC k=10 with edge-replicated left pad) with no
nonlinearity between them, applied causally.  The whole map is therefore a
single 38-tap causal conv  out[t] = B + sum_e E[e] @ x[t-e]  (zero-extended
x) plus closed-form boundary corrections for t < 28:

  out[t] += D[t] + [t < 9] * Q[t] @ (G0 @ x[b, 0] - P2_19)

where E, B, D, Q, G0, P2_19 are composed from (w1,b1,w2,b2,wf,bf) on the
host in float64.  This cuts device FLOPs ~100x vs running the three convs.

Sharding: data-parallel over batch, one batch element per NeuronCore
(B=8 = n_cores), weights replicated, no collectives.

Shared device layout (per core):
  xS (128, 4132): host-built, xS[32g+c, 36+tau] = x[b, tau-g, c]
    (4 tap-shifted copies of channel-major x; zero padding built in).
  ew (128, 240) : 10 K-chunk weight tiles, ew[32g+c, 24j+o] = E[4j+g,o,c].

Variants:
  a_f32  — fp32, time-major out (T,24): per 128-t tile, 10 accumulating
           matmuls with the x-window stationary (128 cols -> LDW-bound).
  b_f32r / b_bf16 — channel-major out (24,T), weights stationary (24 cols,
           ~20ns LDW), xS moving at 1 cyc/row: ~3x less PE time. Host
           transposes the (24,T) per-core outputs at gather time.
"""

import os

import numpy as np

B, T, CIN, H, C2, O = 8, 4096, 32, 256, 512, 24
K1, K2, KF = 10, 20, 10
NE = 38          # composed conv taps
NCHUNK = 10      # ceil(NE/4) K-chunks of 128 = 4 taps x 32 channels
OFF = 36         # left halo lookback
W = OFF + T + 28  # xS width, padded so the row stride is 128B-aligned
TILE = 128       # variant a: timesteps per tile
NTILES = T // TILE
TILE_B = 512     # variant b: timesteps per tile (one PSUM bank)
NTILES_B = T // TILE_B
NCORES = 8

VARIANT = os.environ.get("KERNEL_VARIANT", "a_f32")

_cache = {}


def _compose(w1, b1, w2, b2, wf, bf):
    """Compose the three linear stages in float64. Returns
    (E (38,O,CIN), Bconst (O,), D (28,O), Q (9,O,C2), G0 (C2,CIN), P219 (C2,))."""
    w1 = w1.astype(np.float64)
    b1 = b1.astype(np.float64)
    w2 = w2.astype(np.float64)
    b2 = b2.astype(np.float64)
    wf = wf.astype(np.float64)
    bf = bf.astype(np.float64)
    WFk = wf.reshape(O, KF, C2)

    G = np.zeros((29, C2, CIN))
    for k1 in range(K1):
        for k2 in range(K2):
            G[28 - k1 - k2] += w2[:, :, k2] @ w1[:, :, k1]

    E = np.zeros((NE, O, CIN))
    for k in range(KF):
        for d in range(29):
            E[9 - k + d] += WFk[:, k, :] @ G[d]

    hbar = b2 + w2.sum(axis=2) @ b1
    Bconst = bf + WFk.sum(axis=1) @ hbar

    P2 = np.zeros((21, C2))
    for m in range(1, 21):
        P2[m] = P2[m - 1] + w2[:, :, m - 1] @ b1

    D = np.zeros((28, O))
    for t in range(28):
        for k in range(KF):
            j = t - 9 + k
            if 0 <= j < 19:
                D[t] -= WFk[:, k, :] @ P2[19 - j]

    Q = np.zeros((9, O, C2))
    for t in range(9):
        Q[t] = WFk[:, : 9 - t, :].sum(axis=1)

    return E, Bconst, D, Q, G[0], P2[19]


def _np_dtype(variant):
    if variant.endswith("bf16"):
        import ml_dtypes

        return np.dtype(ml_dtypes.bfloat16)
    if variant.endswith("fp16"):
        return np.dtype(np.float16)
    return np.dtype(np.float32)


def _build_program(variant=VARIANT, repeat=1):
    import concourse.bacc as bacc
    import concourse.mybir as mybir
    from concourse.tile import TileContext

    f32 = mybir.dt.float32
    if variant in ("a_f32", "m4_f32"):
        mmdt = f32
    elif variant == "b_f32r":
        mmdt = mybir.dt.float32r
    elif variant == "b_bf16":
        mmdt = mybir.dt.bfloat16
    elif variant == "b_fp16":
        mmdt = mybir.dt.float16
    else:
        raise ValueError(variant)

    nc = bacc.Bacc("TRN2", target_bir_lowering=False, debug=False)
    xs = nc.declare_dram_parameter("xs", [128, W], mmdt, isOutput=False)
    ew = nc.declare_dram_parameter("ew", [128, NCHUNK * O], mmdt, isOutput=False)

    with TileContext(nc) as tc:
        with (
            tc.tile_pool(name="const", bufs=1) as cpool,
            tc.tile_pool(name="xwp", bufs=4) as xpool,
            tc.tile_pool(name="ps", bufs=8, space="PSUM") as pspool,
            tc.tile_pool(name="ot", bufs=4) as opool,
        ):
            ew_sb = cpool.tile([128, NCHUNK * O], mmdt)
            nc.sync.dma_start(out=ew_sb[:, :], in_=ew[:, :])
            if variant != "a_f32":
                # whole shifted-x image stays resident in SBUF (1-2 MB)
                xs_sb = cpool.tile([128, W], mmdt)
                nc.sync.dma_start(out=xs_sb[:, :], in_=xs[:, :])

            if variant == "a_f32":
                # LDWEIGHTS from a wide resident tile measured 2.4x slower, so
                # stage compact per-tile windows via DMA instead.
                bias = nc.declare_dram_parameter("bias", [2 * 128, O], f32,
                                                 isOutput=False)
                out = nc.declare_dram_parameter("out", [T, O], f32, isOutput=True)
                bias0 = cpool.tile([128, O], f32)
                biasR = cpool.tile([128, O], f32)
                nc.sync.dma_start(out=bias0[:, :], in_=bias[0:128, :])
                nc.sync.dma_start(out=biasR[:, :], in_=bias[128:256, :])

                def body():
                    for i in range(NTILES):
                        t0 = i * TILE
                        xw = xpool.tile([128, OFF + TILE], f32)
                        nc.sync.dma_start(
                            out=xw[:, :], in_=xs[:, t0 : t0 + OFF + TILE]
                        )
                        ps = pspool.tile([128, O], f32)
                        for j in range(NCHUNK):
                            lo = OFF - 4 * j
                            nc.tensor.matmul(
                                out=ps[:, :],
                                lhsT=xw[:, lo : lo + 128],
                                rhs=ew_sb[:, j * O : (j + 1) * O],
                                start=(j == 0),
                                stop=(j == NCHUNK - 1),
                            )
                        ot = opool.tile([128, O], f32)
                        nc.vector.tensor_add(
                            out=ot[:, :],
                            in0=ps[:, :],
                            in1=(bias0 if i == 0 else biasR)[:, :],
                        )
                        nc.sync.dma_start(out=out[t0 : t0 + TILE, :], in_=ot[:, :])

            elif variant == "m4_f32":
                # fp32-exact, 4x column-tiled: 4 weight chunks stream
                # concurrently in disjoint 32-col PE strips; partials land in
                # 4 partition strips of one PSUM bank; a stacked-identity
                # fp32 matmul reduces the strips. out channel-major (24, T).
                bias = nc.declare_dram_parameter("bias", [2 * O, TILE_B], f32,
                                                 isOutput=False)
                idr = nc.declare_dram_parameter("idr", [128, O], f32,
                                                isOutput=False)
                out = nc.declare_dram_parameter("out", [O, T], f32, isOutput=True)
                bias0 = cpool.tile([O, TILE_B], f32)
                biasR = cpool.tile([O, TILE_B], f32)
                nc.sync.dma_start(out=bias0[:, :], in_=bias[0:O, :])
                nc.sync.dma_start(out=biasR[:, :], in_=bias[O : 2 * O, :])
                idr_sb = cpool.tile([128, O], f32)
                nc.sync.dma_start(out=idr_sb[:, :], in_=idr[:, :])
                # staging tile for PSUM->SBUF strip copies; zeroed once so the
                # 8-row bands between strips stay 0 for the reduce matmul
                cp = cpool.tile([128, TILE_B], f32)
                nc.any.memset(cp[:, :], 0.0)

                def body():
                    for i in range(NTILES_B):
                        t0 = i * TILE_B
                        ps = pspool.tile([128, TILE_B], f32, tag="psbank", bufs=3)
                        # waves: (j=0..3 on strips 0..3), (4..7), (8..9)
                        for g in range(3):
                            strips = range(4) if g < 2 else range(2)
                            for s in strips:
                                j = 4 * g + s
                                lo = t0 + OFF - 4 * j
                                nc.tensor.matmul(
                                    out=ps[32 * s : 32 * s + O, :],
                                    lhsT=ew_sb[:, j * O : (j + 1) * O],
                                    rhs=xs_sb[:, lo : lo + TILE_B],
                                    start=(g == 0),
                                    stop=(g == 2) or (g == 1 and s >= 2),
                                    tile_position=(0, 32 * s),
                                    skip_group_check=True,
                                )
                        for s in range(4):
                            nc.vector.tensor_copy(
                                out=cp[32 * s : 32 * s + O, :],
                                in_=ps[32 * s : 32 * s + O, :],
                            )
                        ps2 = pspool.tile([O, TILE_B], f32, tag="psred", bufs=3)
                        nc.tensor.matmul(
                            out=ps2[:, :], lhsT=idr_sb[:, :], rhs=cp[:, :],
                            start=True, stop=True,
                        )
                        ot = opool.tile([O, TILE_B], f32)
                        nc.vector.tensor_add(
                            out=ot[:, :],
                            in0=ps2[:, :],
                            in1=(bias0 if i == 0 else biasR)[:, :],
                        )
                        nc.sync.dma_start(
                            out=out[:, t0 : t0 + TILE_B], in_=ot[:, :]
                        )

            else:
                # channel-major: out_cm (24, T); bias blocks (24, TILE_B) x2
                bias = nc.declare_dram_parameter("bias", [2 * O, TILE_B], f32,
                                                 isOutput=False)
                out = nc.declare_dram_parameter("out", [O, T], f32, isOutput=True)
                bias0 = cpool.tile([O, TILE_B], f32)
                biasR = cpool.tile([O, TILE_B], f32)
                nc.sync.dma_start(out=bias0[:, :], in_=bias[0:O, :])
                nc.sync.dma_start(out=biasR[:, :], in_=bias[O : 2 * O, :])

                def body():
                    for i in range(NTILES_B):
                        t0 = i * TILE_B
                        ps = pspool.tile([O, TILE_B], f32)
                        for j in range(NCHUNK):
                            lo = t0 + OFF - 4 * j
                            nc.tensor.matmul(
                                out=ps[:, :],
                                lhsT=ew_sb[:, j * O : (j + 1) * O],
                                rhs=xs_sb[:, lo : lo + TILE_B],
                                start=(j == 0),
                                stop=(j == NCHUNK - 1),
                            )
                        ot = opool.tile([O, TILE_B], f32)
                        nc.vector.tensor_add(
                            out=ot[:, :],
                            in0=ps[:, :],
                            in1=(bias0 if i == 0 else biasR)[:, :],
                        )
                        nc.sync.dma_start(
                            out=out[:, t0 : t0 + TILE_B], in_=ot[:, :]
                        )

            if repeat == 1:
                body()
            else:
                hints = (
                    mybir.EngineType.PE,
                    mybir.EngineType.SP,
                    mybir.EngineType.DVE,
                    mybir.EngineType.Activation,
                )
                with tc.For_i(0, repeat, 1, hint_engines=hints):
                    body()
    nc.compile()
    return nc


def _prep_in_maps(inputs, variant=VARIANT):
    x = np.ascontiguousarray(np.asarray(inputs["x"], dtype=np.float32))
    E, Bconst, D, Q, G0, P219 = _compose(
        np.asarray(inputs["w1"]), np.asarray(inputs["b1"]),
        np.asarray(inputs["w2"]), np.asarray(inputs["b2"]),
        np.asarray(inputs["wf"]), np.asarray(inputs["bf"]),
    )
    ndt = _np_dtype(variant)

    # ew: (128, 240): ew[32g + c, 24j + o] = E[4j+g, o, c] (zero for e >= 38)
    Epad = np.zeros((40, O, CIN))
    Epad[:NE] = E
    ew = np.ascontiguousarray(
        Epad.reshape(NCHUNK, 4, O, CIN)          # (j, g, o, c)
        .transpose(1, 3, 0, 2)                   # (g, c, j, o)
        .reshape(128, NCHUNK * O)
        .astype(ndt)
    )

    # xS per core: (128, W), xS[32g+c, OFF+g+r] = x[b, r, c]
    xS = np.zeros((B, 128, W), dtype=ndt)
    xT = x.transpose(0, 2, 1).astype(ndt)  # (B, CIN, T)
    for g in range(4):
        n = min(T, W - OFF - g)
        xS[:, 32 * g : 32 * g + 32, OFF + g : OFF + g + n] = xT[:, :, :n]

    # per-core per-timestep bias (fp32): corr[t] for t < 28, else Bconst
    corr = np.zeros((B, 28, O))
    for b in range(B):
        v = G0 @ x[b, 0].astype(np.float64) - P219
        corr[b] = D + Bconst
        corr[b, :9] += Q @ v

    if variant == "a_f32":
        bias_all = np.empty((B, 2 * 128, O), dtype=np.float32)
        for b in range(B):
            bias_all[b] = np.broadcast_to(Bconst, (256, O))
            bias_all[b, :28] = corr[b]
    else:
        bias_all = np.empty((B, 2 * O, TILE_B), dtype=np.float32)
        for b in range(B):
            bias_all[b] = np.tile(Bconst[:, None], (2, TILE_B))
            bias_all[b, :O, :28] = corr[b].T

    maps = [
        {"xs": np.ascontiguousarray(xS[b]), "ew": ew,
         "bias": np.ascontiguousarray(bias_all[b])}
        for b in range(B)
    ]
    if variant == "m4_f32":
        idr = np.zeros((128, O), dtype=np.float32)
        for s in range(4):
            idr[32 * s + np.arange(O), np.arange(O)] = 1.0
        for m in maps:
            m["idr"] = idr
    return maps


def _get_program(variant=VARIANT, repeat=1):
    key = (variant, repeat)
    if key not in _cache:
        _cache[key] = _build_program(variant, repeat)
    return _cache[key]


def _gather(results, variant=VARIANT):
    out = np.stack([np.asarray(results[b]["out"]) for b in range(B)])
    if variant != "a_f32":
        out = np.ascontiguousarray(out.transpose(0, 2, 1))
    return out.astype(np.float32, copy=False)


def _run(inputs, variant=VARIANT, trace=False, **spmd_kwargs):
    from concourse.bass_utils import run_bass_kernel_spmd

    nc = _get_program(variant)
    in_maps = _prep_in_maps(inputs, variant)
    res = run_bass_kernel_spmd(
        nc, in_maps, list(range(NCORES)), trace=trace, **spmd_kwargs
    )
    return _gather(res.results, variant), res


def kernel(**inputs) -> np.ndarray:
    out, _ = _run(inputs, trace=False)
    return out


# revision 27
# speedup vs baseline: 6.3427x; 6.3427x over previous
"""Trainium2 Bass kernel for nn_CNN_25744033972549.

The reference network is three *linear* stages (conv k=10 pad=9, conv k=20
pad=19, sliding-window FC k=10 with edge-replicated left pad) with no
nonlinearity between them, applied causally.  The whole map is therefore a
single 38-tap causal conv  out[t] = B + sum_e E[e] @ x[t-e]  (zero-extended
x) plus closed-form boundary corrections for t < 28:

  out[t] += D[t] + [t < 9] * Q[t] @ (G0 @ x[b, 0] - P2_19)

where E, B, D, Q, G0, P2_19 are composed from (w1,b1,w2,b2,wf,bf) on the
host in float64.  This cuts device FLOPs ~100x vs running the three convs.

Sharding: data-parallel over batch, one batch element per NeuronCore
(B=8 = n_cores), weights replicated, no collectives.

Shared device layout (per core):
  xS (128, 4132): host-built, xS[32g+c, 36+tau] = x[b, tau-g, c]
    (4 tap-shifted copies of channel-major x; zero padding built in).
  ew (128, 240) : 10 K-chunk weight tiles, ew[32g+c, 24j+o] = E[4j+g,o,c].

Variants:
  a_f32  — fp32, time-major out (T,24): per 128-t tile, 10 accumulating
           matmuls with the x-window stationary (128 cols -> LDW-bound).
  b_f32r / b_bf16 — channel-major out (24,T), weights stationary (24 cols,
           ~20ns LDW), xS moving at 1 cyc/row: ~3x less PE time. Host
           transposes the (24,T) per-core outputs at gather time.
"""

import os

import numpy as np

B, T, CIN, H, C2, O = 8, 4096, 32, 256, 512, 24
K1, K2, KF = 10, 20, 10
NE = 38          # composed conv taps
NCHUNK = 10      # ceil(NE/4) K-chunks of 128 = 4 taps x 32 channels
OFF = 36         # left halo lookback
W = OFF + T      # xS width
TILE = 128       # variant a: timesteps per tile
NTILES = T // TILE
TILE_B = 512     # variant b: timesteps per tile (one PSUM bank)
NTILES_B = T // TILE_B
NCORES = 8

VARIANT = os.environ.get("KERNEL_VARIANT", "a_f32")

_cache = {}


def _compose(w1, b1, w2, b2, wf, bf):
    """Compose the three linear stages in float64. Returns
    (E (38,O,CIN), Bconst (O,), D (28,O), Q (9,O,C2), G0 (C2,CIN), P219 (C2,))."""
    w1 = w1.astype(np.float64)
    b1 = b1.astype(np.float64)
    w2 = w2.astype(np.float64)
    b2 = b2.astype(np.float64)
    wf = wf.astype(np.float64)
    bf = bf.astype(np.float64)
    WFk = wf.reshape(O, KF, C2)

    G = np.zeros((29, C2, CIN))
    for k1 in range(K1):
        for k2 in range(K2):
            G[28 - k1 - k2] += w2[:, :, k2] @ w1[:, :, k1]

    E = np.zeros((NE, O, CIN))
    for k in range(KF):
        for d in range(29):
            E[9 - k + d] += WFk[:, k, :] @ G[d]

    hbar = b2 + w2.sum(axis=2) @ b1
    Bconst = bf + WFk.sum(axis=1) @ hbar

    P2 = np.zeros((21, C2))
    for m in range(1, 21):
        P2[m] = P2[m - 1] + w2[:, :, m - 1] @ b1

    D = np.zeros((28, O))
    for t in range(28):
        for k in range(KF):
            j = t - 9 + k
            if 0 <= j < 19:
                D[t] -= WFk[:, k, :] @ P2[19 - j]

    Q = np.zeros((9, O, C2))
    for t in range(9):
        Q[t] = WFk[:, : 9 - t, :].sum(axis=1)

    return E, Bconst, D, Q, G[0], P2[19]


def _np_dtype(variant):
    if variant.endswith("bf16"):
        import ml_dtypes

        return np.dtype(ml_dtypes.bfloat16)
    if variant.endswith("fp16"):
        return np.dtype(np.float16)
    return np.dtype(np.float32)


def _build_program(variant=VARIANT, repeat=1):
    import concourse.bacc as bacc
    import concourse.mybir as mybir
    from concourse.tile import TileContext

    f32 = mybir.dt.float32
    if variant in ("a_f32", "m4_f32"):
        mmdt = f32
    elif variant == "b_f32r":
        mmdt = mybir.dt.float32r
    elif variant == "b_bf16":
        mmdt = mybir.dt.bfloat16
    elif variant == "b_fp16":
        mmdt = mybir.dt.float16
    else:
        raise ValueError(variant)

    nc = bacc.Bacc("TRN2", target_bir_lowering=False, debug=False)
    xs = nc.declare_dram_parameter("xs", [128, W], mmdt, isOutput=False)
    ew = nc.declare_dram_parameter("ew", [128, NCHUNK * O], mmdt, isOutput=False)

    with TileContext(nc) as tc:
        with (
            tc.tile_pool(name="const", bufs=1) as cpool,
            tc.tile_pool(name="xwp", bufs=4) as xpool,
            tc.tile_pool(name="ps", bufs=8, space="PSUM") as pspool,
            tc.tile_pool(name="ot", bufs=4) as opool,
        ):
            ew_sb = cpool.tile([128, NCHUNK * O], mmdt)
            nc.sync.dma_start(out=ew_sb[:, :], in_=ew[:, :])
            if variant != "a_f32":
                # whole shifted-x image stays resident in SBUF (1-2 MB)
                xs_sb = cpool.tile([128, W], mmdt)
                nc.sync.dma_start(out=xs_sb[:, :], in_=xs[:, :])

            if variant == "a_f32":
                # LDWEIGHTS from a wide resident tile measured 2.4x slower, so
                # stage compact per-tile windows via DMA instead.
                bias = nc.declare_dram_parameter("bias", [2 * 128, O], f32,
                                                 isOutput=False)
                out = nc.declare_dram_parameter("out", [T, O], f32, isOutput=True)
                bias0 = cpool.tile([128, O], f32)
                biasR = cpool.tile([128, O], f32)
                nc.sync.dma_start(out=bias0[:, :], in_=bias[0:128, :])
                nc.sync.dma_start(out=biasR[:, :], in_=bias[128:256, :])

                def body():
                    for i in range(NTILES):
                        t0 = i * TILE
                        xw = xpool.tile([128, OFF + TILE], f32)
                        nc.sync.dma_start(
                            out=xw[:, :], in_=xs[:, t0 : t0 + OFF + TILE]
                        )
                        ps = pspool.tile([128, O], f32, bufs=4)
                        for j in range(NCHUNK):
                            lo = OFF - 4 * j
                            nc.tensor.matmul(
                                out=ps[:, :],
                                lhsT=xw[:, lo : lo + 128],
                                rhs=ew_sb[:, j * O : (j + 1) * O],
                                start=(j == 0),
                                stop=(j == NCHUNK - 1),
                            )
                        ot = opool.tile([128, O], f32)
                        nc.vector.tensor_add(
                            out=ot[:, :],
                            in0=ps[:, :],
                            in1=(bias0 if i == 0 else biasR)[:, :],
                        )
                        nc.sync.dma_start(out=out[t0 : t0 + TILE, :], in_=ot[:, :])

            elif variant == "m4_f32":
                # fp32-exact, 4x column-tiled: 4 weight chunks stream
                # concurrently in disjoint 32-col PE strips; partials land in
                # 4 partition strips of one PSUM bank; a stacked-identity
                # fp32 matmul reduces the strips. out channel-major (24, T).
                bias = nc.declare_dram_parameter("bias", [2 * O, TILE_B], f32,
                                                 isOutput=False)
                idr = nc.declare_dram_parameter("idr", [128, O], f32,
                                                isOutput=False)
                out = nc.declare_dram_parameter("out", [O, T], f32, isOutput=True)
                bias0 = cpool.tile([O, TILE_B], f32)
                biasR = cpool.tile([O, TILE_B], f32)
                nc.sync.dma_start(out=bias0[:, :], in_=bias[0:O, :])
                nc.sync.dma_start(out=biasR[:, :], in_=bias[O : 2 * O, :])
                idr_sb = cpool.tile([128, O], f32)
                nc.sync.dma_start(out=idr_sb[:, :], in_=idr[:, :])
                # staging tile for PSUM->SBUF strip copies; zeroed once so the
                # 8-row bands between strips stay 0 for the reduce matmul
                cp = cpool.tile([128, TILE_B], f32)
                nc.any.memset(cp[:, :], 0.0)

                def body():
                    for i in range(NTILES_B):
                        t0 = i * TILE_B
                        ps = pspool.tile([128, TILE_B], f32, tag="psbank", bufs=3)
                        # waves: (j=0..3 on strips 0..3), (4..7), (8..9)
                        for g in range(3):
                            strips = range(4) if g < 2 else range(2)
                            for s in strips:
                                j = 4 * g + s
                                lo = t0 + OFF - 4 * j
                                nc.tensor.matmul(
                                    out=ps[32 * s : 32 * s + O, :],
                                    lhsT=ew_sb[:, j * O : (j + 1) * O],
                                    rhs=xs_sb[:, lo : lo + TILE_B],
                                    start=(g == 0),
                                    stop=(g == 2) or (g == 1 and s >= 2),
                                    tile_position=(0, 32 * s),
                                    skip_group_check=True,
                                )
                        for s in range(4):
                            nc.vector.tensor_copy(
                                out=cp[32 * s : 32 * s + O, :],
                                in_=ps[32 * s : 32 * s + O, :],
                            )
                        ps2 = pspool.tile([O, TILE_B], f32, tag="psred", bufs=3)
                        nc.tensor.matmul(
                            out=ps2[:, :], lhsT=idr_sb[:, :], rhs=cp[:, :],
                            start=True, stop=True,
                        )
                        ot = opool.tile([O, TILE_B], f32)
                        nc.vector.tensor_add(
                            out=ot[:, :],
                            in0=ps2[:, :],
                            in1=(bias0 if i == 0 else biasR)[:, :],
                        )
                        nc.sync.dma_start(
                            out=out[:, t0 : t0 + TILE_B], in_=ot[:, :]
                        )

            else:
                # channel-major: out_cm (24, T); bias blocks (24, TILE_B) x2
                bias = nc.declare_dram_parameter("bias", [2 * O, TILE_B], f32,
                                                 isOutput=False)
                out = nc.declare_dram_parameter("out", [O, T], f32, isOutput=True)
                bias0 = cpool.tile([O, TILE_B], f32)
                biasR = cpool.tile([O, TILE_B], f32)
                nc.sync.dma_start(out=bias0[:, :], in_=bias[0:O, :])
                nc.sync.dma_start(out=biasR[:, :], in_=bias[O : 2 * O, :])

                def body():
                    for i in range(NTILES_B):
                        t0 = i * TILE_B
                        ps = pspool.tile([O, TILE_B], f32)
                        for j in range(NCHUNK):
                            lo = t0 + OFF - 4 * j
                            nc.tensor.matmul(
                                out=ps[:, :],
                                lhsT=ew_sb[:, j * O : (j + 1) * O],
                                rhs=xs_sb[:, lo : lo + TILE_B],
                                start=(j == 0),
                                stop=(j == NCHUNK - 1),
                            )
                        ot = opool.tile([O, TILE_B], f32)
                        nc.vector.tensor_add(
                            out=ot[:, :],
                            in0=ps[:, :],
                            in1=(bias0 if i == 0 else biasR)[:, :],
                        )
                        nc.sync.dma_start(
                            out=out[:, t0 : t0 + TILE_B], in_=ot[:, :]
                        )

            if repeat == 1:
                body()
            else:
                hints = (
                    mybir.EngineType.PE,
                    mybir.EngineType.SP,
                    mybir.EngineType.DVE,
                    mybir.EngineType.Activation,
                )
                with tc.For_i(0, repeat, 1, hint_engines=hints):
                    body()
    nc.compile()
    return nc


def _prep_in_maps(inputs, variant=VARIANT):
    x = np.ascontiguousarray(np.asarray(inputs["x"], dtype=np.float32))
    E, Bconst, D, Q, G0, P219 = _compose(
        np.asarray(inputs["w1"]), np.asarray(inputs["b1"]),
        np.asarray(inputs["w2"]), np.asarray(inputs["b2"]),
        np.asarray(inputs["wf"]), np.asarray(inputs["bf"]),
    )
    ndt = _np_dtype(variant)

    # ew: (128, 240): ew[32g + c, 24j + o] = E[4j+g, o, c] (zero for e >= 38)
    Epad = np.zeros((40, O, CIN))
    Epad[:NE] = E
    ew = np.ascontiguousarray(
        Epad.reshape(NCHUNK, 4, O, CIN)          # (j, g, o, c)
        .transpose(1, 3, 0, 2)                   # (g, c, j, o)
        .reshape(128, NCHUNK * O)
        .astype(ndt)
    )

    # xS per core: (128, W), xS[32g+c, OFF+g+r] = x[b, r, c]
    xS = np.zeros((B, 128, W), dtype=ndt)
    xT = x.transpose(0, 2, 1).astype(ndt)  # (B, CIN, T)
    for g in range(4):
        n = min(T, W - OFF - g)
        xS[:, 32 * g : 32 * g + 32, OFF + g : OFF + g + n] = xT[:, :, :n]

    # per-core per-timestep bias (fp32): corr[t] for t < 28, else Bconst
    corr = np.zeros((B, 28, O))
    for b in range(B):
        v = G0 @ x[b, 0].astype(np.float64) - P219
        corr[b] = D + Bconst
        corr[b, :9] += Q @ v

    if variant == "a_f32":
        bias_all = np.empty((B, 2 * 128, O), dtype=np.float32)
        for b in range(B):
            bias_all[b] = np.broadcast_to(Bconst, (256, O))
            bias_all[b, :28] = corr[b]
    else:
        bias_all = np.empty((B, 2 * O, TILE_B), dtype=np.float32)
        for b in range(B):
            bias_all[b] = np.tile(Bconst[:, None], (2, TILE_B))
            bias_all[b, :O, :28] = corr[b].T

    maps = [
        {"xs": np.ascontiguousarray(xS[b]), "ew": ew,
         "bias": np.ascontiguousarray(bias_all[b])}
        for b in range(B)
    ]
    if variant == "m4_f32":
        idr = np.zeros((128, O), dtype=np.float32)
        for s in range(4):
            idr[32 * s + np.arange(O), np.arange(O)] = 1.0
        for m in maps:
            m["idr"] = idr
    return maps


def _get_program(variant=VARIANT, repeat=1):
    key = (variant, repeat)
    if key not in _cache:
        _cache[key] = _build_program(variant, repeat)
    return _cache[key]


def _gather(results, variant=VARIANT):
    out = np.stack([np.asarray(results[b]["out"]) for b in range(B)])
    if variant != "a_f32":
        out = np.ascontiguousarray(out.transpose(0, 2, 1))
    return out.astype(np.float32, copy=False)


def _run(inputs, variant=VARIANT, trace=False, **spmd_kwargs):
    from concourse.bass_utils import run_bass_kernel_spmd

    nc = _get_program(variant)
    in_maps = _prep_in_maps(inputs, variant)
    res = run_bass_kernel_spmd(
        nc, in_maps, list(range(NCORES)), trace=trace, **spmd_kwargs
    )
    return _gather(res.results, variant), res


def kernel(**inputs) -> np.ndarray:
    out, _ = _run(inputs, trace=False)
    return out
